# revision 1
# baseline (speedup 1.0000x reference)
"""MDTA (channel attention) kernel for 8 Trainium2 NeuronCores.

Strategy: data-parallel over the 16 independent (batch, head) channel-attention
problems -> 2 per core, combined with a tensor-parallel split of the qkv /
project_out channel dimensions (each core owns the 3*96 qkv channels of its two
heads and contributes a rank-96 partial product to project_out; partials are
summed on gather).  All heavy compute (1x1 convs as GEMMs, depthwise 3x3,
l2-norm, 48x48 channel attention, projection) runs on the NeuronCores via a
shard_map program compiled per-device; host only shards inputs and sums the
8 partial outputs (the project_out all-reduce of the sharding hint, done at
unshard time).
"""

import functools
import numpy as np

import jax
import jax.numpy as jnp
from jax.sharding import Mesh, PartitionSpec as P
from jax.experimental.shard_map import shard_map

B, DIM, HGT, WID = 2, 384, 256, 256
HEADS = 8
HEAD_DIM = DIM // HEADS  # 48
N_CORES = 8
HEADS_PER_CORE = 2  # 16 (b,h) problems / 8 cores
CH = 3 * HEAD_DIM * HEADS_PER_CORE  # 288 qkv channels owned per core


def _dw_conv3x3(x, w):
    # x: [C, H, W]; w: [C, 3, 3]; stride 1, zero pad 1, depthwise.
    xp = jnp.pad(x, ((0, 0), (1, 1), (1, 1)))
    out = jnp.zeros_like(x)
    for di in range(3):
        for dj in range(3):
            out = out + w[:, di, dj][:, None, None] * xp[:, di:di + HGT, dj:dj + WID]
    return out


def _per_core(x_b, qkv_w_c, dw_w_c, proj_w_c, temp_c):
    """One core's program.

    x_b:      [DIM, HGT, WID]      its batch image (fp32)
    qkv_w_c:  [CH, DIM]            rows for its 2 heads' q,k,v channels
    dw_w_c:   [CH, 3, 3]           depthwise filters for those channels
    proj_w_c: [DIM, 2, HEAD_DIM]   proj_w columns for its 2 heads
    temp_c:   [2]                  softplus(log_temp)+eps for its heads
    returns   [DIM, HGT, WID]      partial project_out contribution
    """
    n = HGT * WID
    # 1x1 conv (channel GEMM): [CH, DIM] @ [DIM, N]
    qkv = qkv_w_c @ x_b.reshape(DIM, n)
    # depthwise 3x3
    qkv = _dw_conv3x3(qkv.reshape(CH, HGT, WID), dw_w_c).reshape(CH, n)
    # channel order is [q_h0,q_h1 | k_h0,k_h1 | v_h0,v_h1], 96 each
    q = qkv[0:96].reshape(2, HEAD_DIM, n)
    k = qkv[96:192].reshape(2, HEAD_DIM, n)
    v = qkv[192:288].reshape(2, HEAD_DIM, n)

    qn = q * jax.lax.rsqrt(jnp.maximum((q * q).sum(-1, keepdims=True), 1e-24))
    kn = k * jax.lax.rsqrt(jnp.maximum((k * k).sum(-1, keepdims=True), 1e-24))

    attn = jnp.einsum('hcn,hdn->hcd', qn, kn) * temp_c[:, None, None]
    attn = jax.nn.softmax(attn, axis=-1)

    out = jnp.einsum('hcd,hdn->hcn', attn, v)  # [2, 48, N]
    # partial projection: proj_w[:, core's 96 channels] @ out
    part = jnp.einsum('ohc,hcn->on', proj_w_c, out)  # [DIM, N]
    return part.reshape(DIM, HGT, WID)


@functools.cache
def _build():
    devs = jax.devices()[:N_CORES]
    mesh = Mesh(np.asarray(devs), ('c',))

    def body(x4, q3, d4, p4, t2):
        # local shards have a leading [1] core axis
        return _per_core(x4[0], q3[0], d4[0], p4[0], t2[0])[None]

    run = shard_map(body, mesh=mesh, in_specs=(P('c'),) * 5,
                    out_specs=P('c'), check_rep=False)
    return jax.jit(run)


def kernel(x, qkv_w, dw_w, proj_w, log_temp):
    x = np.asarray(x, np.float32)
    qkv_w = np.asarray(qkv_w, np.float32)
    dw_w = np.asarray(dw_w, np.float32).reshape(3 * DIM, 3, 3)
    proj_w = np.asarray(proj_w, np.float32)
    temp = np.log1p(np.exp(np.asarray(log_temp, np.float32).reshape(HEADS))) + 1e-6

    # --- shard on host -----------------------------------------------------
    xs = np.empty((N_CORES, DIM, HGT, WID), np.float32)
    qw = np.empty((N_CORES, CH, DIM), np.float32)
    dw = np.empty((N_CORES, CH, 3, 3), np.float32)
    pw = np.empty((N_CORES, DIM, HEADS_PER_CORE, HEAD_DIM), np.float32)
    tc = np.empty((N_CORES, HEADS_PER_CORE), np.float32)
    for c in range(N_CORES):
        b = c // 4
        h0 = 2 * (c % 4)
        xs[c] = x[b]
        rows = []
        for sec in range(3):  # q, k, v sections of qkv_w
            for h in (h0, h0 + 1):
                lo = sec * DIM + h * HEAD_DIM
                rows.append(np.arange(lo, lo + HEAD_DIM))
        rows = np.concatenate(rows)
        qw[c] = qkv_w[rows]
        dw[c] = dw_w[rows]
        for i, h in enumerate((h0, h0 + 1)):
            pw[c, :, i] = proj_w[:, h * HEAD_DIM:(h + 1) * HEAD_DIM]
            tc[c, i] = temp[h]

    run = _build()
    parts = np.asarray(jax.block_until_ready(run(xs, qw, dw, pw, tc)))

    # --- gather/unshard: all-reduce of project_out partials per batch ------
    out = np.empty((B, DIM, HGT, WID), np.float32)
    out[0] = parts[0:4].sum(0)
    out[1] = parts[4:8].sum(0)
    return out

